# revision 23
# baseline (speedup 1.0000x reference)
"""CTRNN (6 unfolds) Trainium2 Bass kernel, data-parallel over 8 NeuronCores.

Math (per reference):
    w_x = fc_w[:, :512]; w_h = fc_w[:, 512:]
    xwb = x @ w_x^T + b
    repeat 6x:  f = tanh(xwb + h @ w_h^T);  h = 0.9*h + 0.1*f

Device algorithm (per core: batch shard of 2048, processed as 2 chunk-pairs
of 1024, everything transposed to [feature, batch]):
  State s_t = 10*(xwb + w_h h_t) in bf16; f_t = tanh(0.1*s_t) written
  directly as fp8e4 by the scalar engine.  Update per step:
      psum = w_h f_t (+ xwb for pair 0, folded in-matmul via an fp8
             identity block over a hi+lo split of xwb)
      s    = 0.9*s + psum            (one DVE scalar_tensor_tensor)
      s   += xwb                     (pair 1 only: bf16 tensor add, 2x mode)
  Recurrent matmuls are fp8e4 DoubleRow (K=256/matmul = 2x MACs per issued
  column); each stationary block is reused across the two chunks of a pair
  so LDWEIGHTS stays off the critical path.  Matmul emission uses a rolling
  window that defers each psum group's last k-pair so no group waits on the
  previous step's tanh tail.
  h accumulates as H_t = h_t/0.9^t in bf16: H += (0.1/0.9^{t+1})*f8, as an
  ACT scale-copy plus adds on the DMA engines (SBUF accum-DMA, pair 0) and
  DVE (pair 1, deferred one step to keep the DVE queue clear).  The final
  output is H_6 in bf16; the host unshard applies 0.9^6 and casts to fp32.
  a_0 comes from a bf16 phase (x@w_x then h0@w_h) pipelined into step 0.
"""

import numpy as np
import ml_dtypes
from contextlib import ExitStack

import concourse.bass as bass
import concourse.tile as tile
import concourse.mybir as mybir
from concourse.bass_utils import run_bass_kernel_spmd


def _patch_tile_drain():
    """The walrus build in this image encodes at most one sync-wait on a
    Drain CTRL instruction; Tile's kernel-tail drain attaches one wait per
    outstanding proc and fails codegen ("Too many sync wait commands").
    Spread those waits across single-wait SP nops, then emit a bare drain."""
    if getattr(tile.TileContext, "_drain_split_patched", False):
        return
    from concourse.vector_clock import ScopedClock

    def _drain_and_barrier(self, tick_clock, wait_clock):
        nc = self.nc
        collector = nc.sync.nop(nofuse=True)
        wait_clock.add_sem_waits(
            collector.ins, ScopedClock({None: tick_clock.global_clock})
        )
        waits = list(collector.ins.sync_info.on_wait)
        del collector.ins.sync_info.on_wait[1:]
        for w in waits[1:]:
            nop = nc.sync.nop(nofuse=True)
            if nop.ins.sync_info is None:
                nop.ins.sync_info = mybir.SyncInfo(on_wait=[], on_update=[])
            nop.ins.sync_info.on_wait.append(w)
        nc.sync.drain()
        nc.all_engine_barrier()
        assert self.sems is not None
        popped = nc._tile_sem_poison_stack.pop()
        assert popped is self._sem_poison
        nc.clear_and_free_semaphores(list(self.sems.allocated().values()))
        nc.all_engine_barrier()

    tile.TileContext._drain_and_barrier = _drain_and_barrier
    tile.TileContext._drain_split_patched = True


_patch_tile_drain()


def _split_excess_waits_json(bir_json):
    """This image's walrus encodes at most ONE sync-wait per instruction
    (setupSyncWait: "Too many sync wait commands").  Tile attaches as many
    waits as deps require.  Hoist all but one wait of each instruction onto
    injected NoOps, placed just before it on the same engine."""
    import json as _json

    js = _json.loads(bir_json)
    n_split = 0
    for fn in js["functions"]:
        for blk in fn["blocks"]:
            out_insts = []
            for inst in blk["instructions"]:
                si = inst.get("sync_info") or {}
                ow = si.get("on_wait") or []
                if len(ow) > 1:
                    for w in ow[:-1]:
                        n_split += 1
                        nop = {
                            "name": f"I-ws{n_split}",
                            "opcode": "NoOp",
                            "engine": inst["engine"],
                            "ins": [],
                            "outs": [],
                            "sync_info": {"on_update": [], "on_wait": [w]},
                        }
                        if "debug" in inst:
                            nop["debug"] = inst["debug"]
                        out_insts.append(nop)
                    si["on_wait"] = [ow[-1]]
                out_insts.append(inst)
            blk["instructions"] = out_insts
    return _json.dumps(js).encode()


def _patch_compile_for_wait_cap():
    import concourse.bass_utils as _bu

    if getattr(_bu, "_wait_split_patched", False):
        return
    _orig = _bu._compile_bir_impl

    def _impl(bir_json, *args, **kwargs):
        return _orig(_split_excess_waits_json(bir_json), *args, **kwargs)

    _bu._compile_bir_impl = _impl
    _bu._wait_split_patched = True


_patch_compile_for_wait_cap()

B, D_IN, D_H = 16384, 512, 1024
N_CORES = 8
BS = B // N_CORES            # 2048 batch rows per core
UNFOLDS = 6
DT = 0.1
DECAY = 0.9                  # 1 - DT/TAU
CH = 512                     # batch chunk (matmul moving free dim)
NCH = BS // CH               # 4 chunks per core
KB = D_H // 128              # 8 hidden-dim feature blocks
KQ = D_H // 256              # 4 DoubleRow k-pair blocks
KX = D_IN // 128             # 4 input-dim k-blocks
F32 = mybir.dt.float32
BF16 = mybir.dt.bfloat16
FP8 = mybir.dt.float8e4
NPBF = ml_dtypes.bfloat16
NPF8 = ml_dtypes.float8_e4m3fn
DR = mybir.MatmulPerfMode.DoubleRow
Tanh = mybir.ActivationFunctionType.Tanh
Ident = mybir.ActivationFunctionType.Identity
ACopy = mybir.ActivationFunctionType.Copy
MUL = mybir.AluOpType.mult
ADD = mybir.AluOpType.add
SUB = mybir.AluOpType.subtract


def build_nc() -> bass.Bass:
    nc = bass.Bass()
    xT = nc.dram_tensor("xT", [D_IN, BS], BF16, kind="ExternalInput")
    hT = nc.dram_tensor("hT", [D_H, BS], BF16, kind="ExternalInput")
    wxT = nc.dram_tensor("wxT", [D_IN, D_H], BF16, kind="ExternalInput")
    whT = nc.dram_tensor("whT", [D_H, D_H], BF16, kind="ExternalInput")
    # fp8 w_h^T pre-packed on host: [ki, (q, p, ko, m)]
    wh8 = nc.dram_tensor("wh8", [128, KQ * KB * 2 * 128], FP8, kind="ExternalInput")
    # fp8 identity for the xwb fold: [ki, (ko, m)], both ko planes = I
    id8 = nc.dram_tensor("id8", [128, 2 * 128], FP8, kind="ExternalInput")
    bias = nc.dram_tensor("bias", [128, KB], F32, kind="ExternalInput")
    out = nc.dram_tensor("out", [D_H, BS], BF16, kind="ExternalOutput")

    NP = NCH // 2         # chunk pairs per core
    PW = 2 * CH           # pair width (1024)

    with tile.TileContext(nc) as tc, ExitStack() as ctx:
        persist = ctx.enter_context(tc.tile_pool(name="persist", bufs=1))
        psum_pool = ctx.enter_context(tc.tile_pool(name="psum", bufs=4, space="PSUM"))

        # --- persistent SBUF state, pair-major [128, (p, c2, n)] ---
        s_sb = [
            persist.tile([128, KB * PW], BF16, name=f"s_sb{u}", tag=f"s_sb{u}")
            for u in range(NP)
        ]
        H_sb = [
            persist.tile([128, KB * PW], BF16, name=f"H_sb{u}", tag=f"H_sb{u}")
            for u in range(NP)
        ]
        # pair 0: xwb as fp8 hi/lo [128, (p, ko, c2, n)], folded via id matmul
        xwb8_0 = persist.tile([128, KB * 2 * PW], FP8, name="xwb8_0", tag="xwb8_0")
        # pair 1: xwb as bf16 [128, (p, c2, n)], folded via DVE tensor adds
        xwbh_1 = persist.tile([128, KB * PW], BF16, name="xwbh_1", tag="xwbh_1")
        wh8_sb = persist.tile([128, KQ * KB * 2 * 128], FP8, name="wh8_sb", tag="wh8_sb")
        id8_sb = persist.tile([128, 2 * 128], FP8, name="id8_sb", tag="id8_sb")
        b_sb = persist.tile([128, KB], F32, name="b_sb", tag="b_sb")

        nc.sync.dma_start(out=b_sb[:], in_=bias[:, :])
        nc.sync.dma_start(out=id8_sb[:], in_=id8[:, :])

        fpool = ctx.enter_context(tc.tile_pool(name="fpool", bufs=2))

        def id_mm(ps, p, cc, start):
            # psum[:, cc*CH:+CH] += xwb_hi + xwb_lo  (fp8 DR identity fold)
            nc.tensor.matmul(
                ps[:, cc * CH:(cc + 1) * CH],
                id8_sb[:].rearrange("x (ko m) -> x ko m", ko=2),
                xwb8_0[:, p * 2 * PW:(p + 1) * 2 * PW]
                    .rearrange("x (ko cn) -> x ko cn", ko=2)
                    [:, :, cc * CH:(cc + 1) * CH],
                start=start, stop=True, perf_mode=DR, skip_group_check=True,
            )

        # per-step elementwise emitters -------------------------------------
        def emit_tanh(f8, u):
            for qq in range(KB // 2):
                nc.scalar.activation(
                    f8[u][:, qq * 2 * PW:(qq + 1) * 2 * PW],
                    s_sb[u][:, qq * 2 * PW:(qq + 1) * 2 * PW],
                    Tanh, bias=0.0, scale=0.1,
                )

        # --- phase A ---
        with tc.tile_pool(name="xpre", bufs=1) as xpool, \
             tc.tile_pool(name="wpre", bufs=1) as wpool, \
             tc.tile_pool(name="xwbpre", bufs=3) as xwbpool:
            wx_sb = wpool.tile([128, KX * D_H], BF16, name="wx_sb", tag="wx_sb")
            wh_sb = wpool.tile([128, KB * D_H], BF16, name="wh_sb", tag="wh_sb")
            x_sbs = [
                xpool.tile([128, KX * PW], BF16, name="x_sb", tag=f"x_sb{u}")
                for u in range(NP)
            ]
            # wx and x pair0 gate the first matmuls; split them across the
            # two DMA queues so they land in parallel
            for kb in range(KX):
                nc.gpsimd.dma_start(
                    out=wx_sb[:, kb * D_H:(kb + 1) * D_H],
                    in_=wxT[kb * 128:(kb + 1) * 128, :],
                )
                nc.gpsimd.dma_start(
                    out=x_sbs[0][:, kb * PW:(kb + 1) * PW],
                    in_=xT[kb * 128:(kb + 1) * 128, 0:PW],
                )
            for u in range(1, NP):
                nc.gpsimd.dma_start(
                    out=x_sbs[u][:].rearrange("p (kb c) -> p kb c", c=PW),
                    in_=xT[:, u * PW:(u + 1) * PW].rearrange("(kb p) c -> p kb c", p=128),
                )
            nc.gpsimd.dma_start(
                out=wh_sb[:].rearrange("p (jb h) -> p jb h", h=D_H),
                in_=whT[:, :].rearrange("(jb p) h -> p jb h", p=128),
            )
            nc.gpsimd.dma_start(out=wh8_sb[:], in_=wh8[:, :])
            for u in range(NP):
                nc.gpsimd.dma_start(
                    out=H_sb[u][:].rearrange("p (jb c) -> p jb c", c=PW),
                    in_=hT[:, u * PW:(u + 1) * PW].rearrange("(jb p) c -> p jb c", p=128),
                )

            # A1: xwb1 = x@wx^T + b
            for u in range(NP):
                for p in range(KB):
                    ps = psum_pool.tile([128, PW], F32, name="ps", tag="ps")
                    for kb in range(KX):
                        for cc in range(2):
                            nc.tensor.matmul(
                                ps[:, cc * CH:(cc + 1) * CH],
                                wx_sb[:, kb * D_H + p * 128: kb * D_H + (p + 1) * 128],
                                x_sbs[u][:, kb * PW + cc * CH: kb * PW + (cc + 1) * CH],
                                start=(kb == 0), stop=(kb == KX - 1),
                                skip_group_check=True,
                            )
                    if u == 0:
                        xwb1 = xwbpool.tile([128, PW], BF16, name="xwb1", tag="xwb1")
                        nc.scalar.activation(xwb1[:], ps[:], Ident,
                                             bias=b_sb[:, p:p + 1], scale=1.0)
                        hi = xwb8_0[:, p * 2 * PW: p * 2 * PW + PW]
                        lo = xwb8_0[:, p * 2 * PW + PW: (p + 1) * 2 * PW]
                        nc.vector.tensor_copy(hi, xwb1[:])
                        nc.vector.tensor_tensor(lo, xwb1[:], hi, op=SUB)
                    else:
                        nc.scalar.activation(
                            xwbh_1[:, p * PW:(p + 1) * PW], ps[:], Ident,
                            bias=b_sb[:, p:p + 1], scale=1.0,
                        )

            # A2 (per pair): s0 = 10*(h0@wh^T + xwb1); pipelined with step-0
            # elementwise so the scalar engine never head-of-line blocks.
            f8_t0 = [
                fpool.tile([128, KB * PW], FP8, name=f"f8_{u}", tag=f"f8_{u}")
                for u in range(NP)
            ]
            for u in range(NP):
                for p in (range(KB) if u == 0 else range(KB - 1, -1, -1)):
                    ps = psum_pool.tile([128, PW], F32, name="ps", tag="ps")
                    for jb in range(KB):
                        for cc in range(2):
                            nc.tensor.matmul(
                                ps[:, cc * CH:(cc + 1) * CH],
                                wh_sb[:, jb * D_H + p * 128: jb * D_H + (p + 1) * 128],
                                H_sb[u][:, jb * PW + cc * CH: jb * PW + (cc + 1) * CH],
                                start=(jb == 0), stop=(u == 1 and jb == KB - 1),
                                skip_group_check=True,
                            )
                    if u == 0:
                        for cc in range(2):
                            id_mm(ps, p, cc, start=False)
                        nc.scalar.activation(
                            s_sb[u][:, p * PW:(p + 1) * PW], ps[:], Ident,
                            bias=0.0, scale=10.0,
                        )
                    else:
                        nc.scalar.activation(
                            s_sb[u][:, p * PW:(p + 1) * PW], ps[:], Ident,
                            bias=0.0, scale=10.0,
                        )
                        s_t = s_sb[u][:, p * PW:(p + 1) * PW]
                        nc.vector.scalar_tensor_tensor(
                            s_t, xwbh_1[:, p * PW:(p + 1) * PW], 10.0, s_t,
                            op0=MUL, op1=ADD,
                        )
                # step-0 tanh + H for this pair, interleaved into phase A
                emit_tanh(f8_t0, u)
                if u == 0:
                    nc.vector.scalar_tensor_tensor(
                        H_sb[0][:], f8_t0[0][:], DT / DECAY, H_sb[0][:],
                        op0=MUL, op1=ADD,
                    )
                # pair1's H update is deferred into step 1's stream

        # --- unfold steps 1..5 ---
        gpool = ctx.enter_context(tc.tile_pool(name="gpool", bufs=2))
        sigma = DECAY
        f8_prev = f8_t0
        pending_h1 = [(f8_t0, DT / (1.0 * DECAY))]  # (f8 tiles, c2) for pair 1
        for t in range(1, UNFOLDS):
            last = t == UNFOLDS - 1
            c2 = DT / (sigma * DECAY)
            f8_cur = [
                fpool.tile([128, KB * PW], FP8, name=f"f8_{u}", tag=f"f8_{u}")
                for u in range(NP)
            ]
            for u in range(NP):
                def w_mms(ps, p, qs, first):
                    for q in qs:
                        off = (q * KB + p) * 256
                        for cc in range(2):
                            nc.tensor.matmul(
                                ps[:, cc * CH:(cc + 1) * CH],
                                wh8_sb[:, off:off + 256]
                                    .rearrange("x (ko m) -> x ko m", ko=2),
                                f8_prev[u][:, 2 * q * PW:(2 * q + 2) * PW]
                                    .rearrange("x (ko cn) -> x ko cn", ko=2)
                                    [:, :, cc * CH:(cc + 1) * CH],
                                start=(first and q == qs[0]),
                                stop=(u == 1 and q == KQ - 1),
                                perf_mode=DR, skip_group_check=True,
                            )

                def close_p(ps, p):
                    # deferred last k-pair + xwb fold + s update for tile p
                    w_mms(ps, p, [KQ - 1], first=False)
                    if u == 0:
                        for cc in range(2):
                            id_mm(ps, p, cc, start=False)
                    s_t = s_sb[u][:, p * PW:(p + 1) * PW]
                    nc.vector.scalar_tensor_tensor(
                        s_t, s_t, DECAY, ps[:], op0=MUL, op1=ADD,
                    )
                    if u == 1:
                        nc.vector.tensor_tensor(
                            s_t, xwbh_1[:, p * PW:(p + 1) * PW], s_t, op=ADD)

                # rolling emission: q0..q2 for tile p, closing tile p-2 in
                # between, so no psum group needs f8's last k-pair until ~4us
                # into the step (avoids a stall on the previous step's tail)
                porder = list(range(KB)) if u == 0 else list(range(KB - 1, -1, -1))
                open_ps = {}
                for i, p in enumerate(porder):
                    ps = psum_pool.tile([128, PW], F32, name="ps", tag="ps")
                    open_ps[p] = ps
                    w_mms(ps, p, list(range(KQ - 1)), first=True)
                    if i >= 2:
                        close_p(open_ps.pop(porder[i - 2]), porder[i - 2])
                for p in porder[-2:]:
                    close_p(open_ps.pop(p), p)
                if u == 0:
                    # flush pair1's H update from the previous step: ACT
                    # scale-copy early, then per-p DVE adds (small slices so
                    # they interleave with this step's s-updates)
                    for f8p, c2p in pending_h1:
                        gp = gpool.tile([128, KB * PW], BF16, name="g", tag="g")
                        nc.scalar.activation(gp[:], f8p[1][:], ACopy,
                                             bias=0.0, scale=float(c2p))
                        for p in range(KB):
                            sl = slice(p * PW, (p + 1) * PW)
                            nc.vector.tensor_tensor(
                                H_sb[1][:, sl], gp[:, sl], H_sb[1][:, sl],
                                op=ADD)
                    pending_h1.clear()
                    if not last:
                        emit_tanh(f8_cur, u)
                        # H pair0: ACT scale-copy + DMA-engine accumulate
                        g = gpool.tile([128, KB * PW], BF16, name="g", tag="g")
                        nc.scalar.activation(g[:], f8_cur[0][:], ACopy,
                                             bias=0.0, scale=float(c2))
                        for p in range(KB):
                            sl = slice(p * PW, (p + 1) * PW)
                            nc.gpsimd.dma_start(out=H_sb[0][:, sl], in_=g[:, sl],
                                                accum_op=ADD)
                    else:
                        # final step: everything per tile so the drain
                        # pipelines; out = H_6 bf16 (host rescales by 0.9^6)
                        g = gpool.tile([128, KB * PW], BF16, name="g", tag="g")
                        for p in range(KB):
                            sl = slice(p * PW, (p + 1) * PW)
                            nc.scalar.activation(f8_cur[0][:, sl],
                                                 s_sb[0][:, sl],
                                                 Tanh, bias=0.0, scale=0.1)
                            nc.scalar.activation(g[:, sl], f8_cur[0][:, sl],
                                                 ACopy, bias=0.0, scale=float(c2))
                            nc.gpsimd.dma_start(out=H_sb[0][:, sl], in_=g[:, sl],
                                                accum_op=ADD)
                            nc.sync.dma_start(
                                out=out[p * 128:(p + 1) * 128, u * PW:(u + 1) * PW],
                                in_=H_sb[0][:, sl],
                            )
                else:
                    if not last:
                        emit_tanh(f8_cur, u)
                        pending_h1.append((f8_cur, c2))
                    else:
                        for p in range(KB):
                            sl = slice(p * PW, (p + 1) * PW)
                            nc.scalar.activation(f8_cur[1][:, sl],
                                                 s_sb[1][:, sl],
                                                 Tanh, bias=0.0, scale=0.1)
                            nc.vector.scalar_tensor_tensor(
                                H_sb[1][:, sl], f8_cur[1][:, sl], float(c2),
                                H_sb[1][:, sl], op0=MUL, op1=ADD,
                            )
                            nc.sync.dma_start(
                                out=out[p * 128:(p + 1) * 128, u * PW:(u + 1) * PW],
                                in_=H_sb[1][:, sl],
                            )
            f8_prev = f8_cur
            sigma *= DECAY
    return nc


_NC_CACHE = {}


def _get_nc() -> bass.Bass:
    if "nc" not in _NC_CACHE:
        _NC_CACHE["nc"] = build_nc()
    return _NC_CACHE["nc"]


def make_in_maps(x, h, fc_w, fc_b):
    x = np.asarray(x, dtype=np.float32)
    h = np.asarray(h, dtype=np.float32)
    fc_w = np.asarray(fc_w, dtype=np.float32)
    fc_b = np.asarray(fc_b, dtype=np.float32)
    xT = np.ascontiguousarray(x.T).astype(NPBF)            # [D_IN, B]
    hT = np.ascontiguousarray(h.T).astype(NPBF)            # [D_H, B]
    wxT = np.ascontiguousarray(fc_w[:, :D_IN].T).astype(NPBF)   # [D_IN, D_H]
    whT_f32 = np.ascontiguousarray(fc_w[:, D_IN:].T)       # [D_H, D_H]
    whT = whT_f32.astype(NPBF)
    # fp8 stationary pack: wh8[ki, q, p, ko, m] = whT8[q*256+ko*128+ki, p*128+m]
    whT8 = whT_f32.astype(NPF8)
    t = whT8.reshape(KQ, 2, 128, KB, 128)                  # [q, ko, ki, p, m]
    wh8 = np.ascontiguousarray(t.transpose(2, 0, 3, 1, 4).reshape(128, -1))
    id8 = np.zeros((128, 2, 128), dtype=NPF8)
    for ki in range(128):
        id8[ki, 0, ki] = 1.0
        id8[ki, 1, ki] = 1.0
    id8 = np.ascontiguousarray(id8.reshape(128, -1))
    bias = np.ascontiguousarray(fc_b.reshape(KB, 128).T)   # [128, KB]
    in_maps = []
    for i in range(N_CORES):
        sl = slice(i * BS, (i + 1) * BS)
        in_maps.append({
            "xT": np.ascontiguousarray(xT[:, sl]),
            "hT": np.ascontiguousarray(hT[:, sl]),
            "wxT": wxT,
            "whT": whT,
            "wh8": wh8,
            "id8": id8,
            "bias": bias,
        })
    return in_maps


def gather_out(results):
    outT = np.concatenate([results[i]["out"] for i in range(N_CORES)], axis=1)
    o = outT.T.astype(np.float32)  # [B, D_H], bf16 -> fp32 on host
    o *= np.float32(DECAY ** UNFOLDS)  # final decay folded out of the kernel
    return np.ascontiguousarray(o)


def kernel(x, h, fc_w, fc_b):
    nc = _get_nc()
    in_maps = make_in_maps(x, h, fc_w, fc_b)
    res = run_bass_kernel_spmd(nc, in_maps, list(range(N_CORES)))
    out = gather_out(res.results)
    return (out, out)


if __name__ == "__main__":
    rng = np.random.default_rng(0)
    x = rng.standard_normal((B, D_IN), dtype=np.float32)
    h = rng.standard_normal((B, D_H), dtype=np.float32)
    fc_w = rng.standard_normal((D_H, D_IN + D_H), dtype=np.float32) / np.sqrt(D_IN + D_H)
    fc_b = np.zeros((D_H,), dtype=np.float32)
    o, _ = kernel(x, h, fc_w, fc_b)
    print(o.shape, o.dtype)
